# revision 10
# baseline (speedup 1.0000x reference)
"""GCN sublayer (3x GCNConv + BN/ReLU + global max pool) on 8 Trainium2 cores.

Sharding: destination nodes are partitioned across the 8 cores (12500 each).
Per conv:
  A) each core computes y = dinv * (h_prev @ W) for its owned nodes
     (node-major chunks straight out of PSUM)
  B) chunked AllGather of y into 4 "group tables" of 25000 rows each
     (25000 < 32768 so dma_gather's int16 indices can address a table)
  C) aggregation: per (window of 512 dest cols) x (source group):
     dma_gather the source rows for the edges, build one-hot S tiles with
     a DVE is_equal against an iota row, and matmul-accumulate M^T @ S into
     a PSUM bank window.  Edges (incl. self loops) are sorted on the host by
     (dest-window, group, dest-block) and padded to 128-edge tiles per
     (group, block); pad slots carry dslot=255 which yields an all-zero
     one-hot row.  A zero "dummy" matmul with start=True clears the bank's
     has_written bits once per window so real matmuls all accumulate.
  D) BN statistics via bn_stats/bn_aggr + a tiny AllReduce; fused affine +
     ReLU (the conv bias cancels under training-mode BN for convs 1 and 2;
     conv 3 adds b3 explicitly).
  E) pooling: PE-transpose h to node-major fp16 in HBM, transpose-mode
     dma_gather into a [segment, lane] slotted layout, one segmented
     tensor_reduce(max).
Host code does all integer preprocessing (degrees, edge sorting, index
construction) and the final cross-core max over boundary segments.
"""

import numpy as np

import concourse.bass as bass
import concourse.tile as tile
from concourse import bacc
from concourse import mybir
from concourse.bass_utils import run_bass_kernel_spmd
from concourse.masks import make_identity

F32 = mybir.dt.float32
F16 = mybir.dt.float16
I16 = mybir.dt.int16

CFG = dict(
    N=100000,
    S=2048,
    ncores=8,
    msg_dt="float32",   # dtype of y / gathered messages / one-hot S
    eps=1e-5,
    subt=16,            # tiles (of 128 edges) per gather/S-build subcall
    cseg=32,            # pooling segments per gather chunk
    nconv=3,            # debug: number of convs to run
    skip_bn=False,      # debug
    skip_pool=False,    # debug
    skip_agg=False,     # debug
)

D = 128        # feature width of every stage
BLK = 128      # dest block (one matmul's output column span)
WIN = 512      # PSUM window = one fp32 bank
GRP = 4        # source groups (tables of N/GRP rows, int16-indexable)
PAD_NEG = -60000.0


def _cdiv(a, b):
    return (a + b - 1) // b


# -------------------------------------------------------------------------
# host-side preprocessing
# -------------------------------------------------------------------------
def host_prep(x, edge_index, node_to_subgraph, cfg):
    N = cfg["N"]; S = cfg["S"]; NC = cfg["ncores"]
    P = N // NC
    QTR = P // GRP
    GSZ = QTR * NC
    assert P * NC == N and QTR * GRP == P and GSZ < 32768
    NWIN = _cdiv(P, WIN)
    NCHK = _cdiv(P, BLK)
    NBLK = NCHK

    src = np.asarray(edge_index[0], dtype=np.int64)
    dst = np.asarray(edge_index[1], dtype=np.int64)
    loops = np.arange(N, dtype=np.int64)
    src = np.concatenate([src, loops])
    dst = np.concatenate([dst, loops])

    deg = np.bincount(dst, minlength=N).astype(np.float64)  # incl. self loop
    dinv = (1.0 / np.sqrt(deg)).astype(np.float32)

    core = dst // P
    dloc = dst - core * P
    w = dloc // WIN
    b = dloc // BLK
    g = (src % P) // QTR
    tbl = QTR * (src // P) + (src % QTR)        # row inside group table

    order = np.lexsort((b, g, w, core))
    core = core[order]; w = w[order]; g = g[order]; b = b[order]
    dloc = dloc[order]; tbl = tbl[order]

    cell = ((core * GRP + g) * NWIN + w) * NBLK + b
    n_cells = NC * GRP * NWIN * NBLK
    counts = np.bincount(cell, minlength=n_cells).reshape(NC, GRP, NWIN, NBLK)
    sched = -(-counts.max(axis=0) // 128)       # [GRP, NWIN, NBLK] tiles
    for wi in range(NWIN):
        lo = wi * WIN // BLK
        hi = _cdiv(min((wi + 1) * WIN, P), BLK)
        sched[:, wi, :lo] = 0
        sched[:, wi, hi:] = 0

    tiles_wgb = np.transpose(sched, (1, 0, 2))  # [NWIN, GRP, NBLK]
    flat = tiles_wgb.reshape(-1)
    tile_base = np.concatenate([[0], np.cumsum(flat)])[:-1].reshape(
        NWIN, GRP, NBLK)
    TOT_TILES = int(flat.sum())
    TOT_SLOTS = TOT_TILES * 128

    # rank of each edge within its (core, g, w, b) cell
    sort2 = np.argsort(cell, kind="stable")
    inv2 = np.empty_like(sort2); inv2[sort2] = np.arange(len(sort2))
    cell_sorted = cell[sort2]
    starts = np.searchsorted(cell_sorted, np.arange(n_cells))
    rank = (np.arange(len(cell)) - starts[cell_sorted])[inv2]

    slot = tile_base[w, g, b] * 128 + rank
    assert slot.max() < TOT_SLOTS

    idx_flat = np.zeros((NC, TOT_SLOTS), dtype=np.int16)
    dslot_flat = np.full((NC, TOT_SLOTS), 255.0, dtype=np.float32)
    idx_flat[core, slot] = tbl.astype(np.int16)
    dslot_flat[core, slot] = (dloc % BLK).astype(np.float32)

    # wrapped int16 layout: arr[p, s] = flat[16*s + p%16], replicated to 128
    iw = np.transpose(idx_flat.reshape(NC, TOT_SLOTS // 16, 16), (0, 2, 1))
    idx_wrapped = np.tile(iw, (1, 8, 1)).copy()
    # dslot layout for S building: arr[p, t] = flat[128*t + p]
    dslot_arr = np.transpose(
        dslot_flat.reshape(NC, TOT_TILES, 128), (0, 2, 1)).copy()

    # ---------------- pooling layout ----------------------------------
    nts = np.asarray(node_to_subgraph, dtype=np.int64)
    seg_counts = np.bincount(nts, minlength=S)
    L = int(seg_counts.max())
    s_first = np.empty(NC, dtype=np.int64)
    nseg = np.empty(NC, dtype=np.int64)
    for c in range(NC):
        lo, hi = c * P, (c + 1) * P
        s_first[c] = nts[lo]
        nseg[c] = nts[hi - 1] - nts[lo] + 1
    SLOC = int(nseg.max())
    CSEG = cfg["cseg"]
    NPCH = _cdiv(SLOC, CSEG)               # pooling gather chunks
    PC = _cdiv(CSEG * L, 128) * 128        # slots per chunk (padded)
    PGRID = NPCH * PC

    pool_idx = np.full((NC, PGRID), P, dtype=np.int16)  # row P = pad row
    for c in range(NC):
        lo = c * P
        loc_seg = nts[lo:lo + P] - s_first[c]
        runs = np.concatenate(
            [[0], np.cumsum(np.bincount(loc_seg, minlength=SLOC))])[:-1]
        lane = np.arange(P) - runs[loc_seg]
        assert lane.max() < L
        ch, sin = np.divmod(loc_seg, CSEG)
        pool_idx[c, ch * PC + sin * L + lane] = np.arange(P, dtype=np.int16)
    pw = np.transpose(pool_idx.reshape(NC, PGRID // 16, 16), (0, 2, 1))
    pool_wrapped = np.tile(pw, (1, 8, 1)).copy()

    # ---------------- numeric per-core tensors ------------------------
    x = np.asarray(x, dtype=np.float32)
    h0T = np.stack([x[c * P:(c + 1) * P].T for c in range(NC)])
    dinv_src = np.zeros((NC, 128, NCHK), dtype=np.float32)
    for c in range(NC):
        padv = np.zeros(NCHK * 128, dtype=np.float32)
        padv[:P] = dinv[c * P:(c + 1) * P]
        dinv_src[c] = padv.reshape(NCHK, 128).T
    dinv_dst = np.stack([dinv[c * P:(c + 1) * P].reshape(1, P)
                         for c in range(NC)])

    meta = dict(
        P=P, QTR=QTR, GSZ=GSZ, NWIN=NWIN, NCHK=NCHK, NBLK=NBLK,
        sched=sched, tile_base=tile_base, TOT_TILES=TOT_TILES,
        TOT_SLOTS=TOT_SLOTS, L=L, SLOC=SLOC, PGRID=PGRID,
        NPCH=NPCH, PC=PC,
        s_first=s_first, nseg=nseg, seg_counts=seg_counts,
    )
    per_core = dict(
        h0T=h0T, idx=idx_wrapped, dslot=dslot_arr, pool_idx=pool_wrapped,
        dinv_src=dinv_src, dinv_dst=dinv_dst,
    )
    return meta, per_core


# -------------------------------------------------------------------------
# bass program
# -------------------------------------------------------------------------
def build_program(meta, cfg):
    N = cfg["N"]; NC = cfg["ncores"]
    P = meta["P"]; QTR = meta["QTR"]; GSZ = meta["GSZ"]
    NWIN = meta["NWIN"]; NCHK = meta["NCHK"]; NBLK = meta["NBLK"]
    sched = meta["sched"]; tile_base = meta["tile_base"]
    TOT_TILES = meta["TOT_TILES"]; TOT_SLOTS = meta["TOT_SLOTS"]
    SLOC = meta["SLOC"]; L = meta["L"]; PGRID = meta["PGRID"]
    NPCH = meta["NPCH"]; PC = meta["PC"]
    SUBT = cfg["subt"]; CSEG = cfg["cseg"]
    MSG = getattr(mybir.dt, cfg["msg_dt"])
    eps = cfg["eps"]

    nc = bacc.Bacc(num_devices=NC)

    h0T_d = nc.declare_dram_parameter("h0T", [128, P], F32, False)
    W_d = [nc.declare_dram_parameter(f"W{k + 1}", [128, 128], F32, False)
           for k in range(3)]
    vec_names = ["g1", "be1", "g2", "be2", "b3"]
    vec_d = {n: nc.declare_dram_parameter(n, [128, 1], F32, False)
             for n in vec_names}
    dinv_src_d = nc.declare_dram_parameter("dinv_src", [128, NCHK], F32, False)
    dinv_dst_d = nc.declare_dram_parameter("dinv_dst", [1, P], F32, False)
    idx_d = nc.declare_dram_parameter("idx", [128, TOT_SLOTS // 16], I16,
                                      False)
    dslot_d = nc.declare_dram_parameter("dslot", [128, TOT_TILES], F32, False)
    pool_idx_d = nc.declare_dram_parameter("pool_idx", [128, PGRID // 16],
                                           I16, False)
    iota_d = nc.declare_dram_parameter("iota", [128, 128], F32, False)
    out_d = nc.declare_dram_parameter("pool_out", [3, 128, SLOC], F32, True)

    rg = [list(range(NC))]

    def bcast_p(ap, parts):
        """broadcast a [1, cols] DRAM AP across partitions (DMA source only)"""
        return bass.AP(tensor=ap.tensor, offset=ap.offset,
                       ap=[[0, parts]] + list(ap.ap[1:]))

    with tile.TileContext(nc) as tc, \
            tc.tile_pool(name="const", bufs=1) as const, \
            tc.tile_pool(name="pre", bufs=2) as prepool, \
            tc.tile_pool(name="ystg", bufs=3) as ystg, \
            tc.tile_pool(name="msg", bufs=3) as msgp, \
            tc.tile_pool(name="sp", bufs=3) as spp, \
            tc.tile_pool(name="ixp", bufs=3) as ixp, \
            tc.tile_pool(name="dw", bufs=2) as dwp, \
            tc.tile_pool(name="small", bufs=4) as small, \
            tc.tile_pool(name="trp", bufs=3) as trp, \
            tc.tile_pool(name="pgp", bufs=3) as pgp, \
            tc.tile_pool(name="ps_y", bufs=2, space="PSUM") as ps_y, \
            tc.tile_pool(name="ps_agg", bufs=3, space="PSUM") as ps_agg, \
            tc.tile_pool(name="ps_tr", bufs=2, space="PSUM") as ps_tr, \
            tc.tile_pool(name="dram", bufs=1, space="DRAM") as dram, \
            tc.tile_pool(name="dram2", bufs=2, space="DRAM") as dram2:

        # ---- constants -------------------------------------------------
        ident = const.tile([128, 128], F32, tag="ident")
        make_identity(nc, ident[:])
        W_sb = []
        for k in range(3):
            t = const.tile([128, 128], F32, tag=f"W{k}", name=f"W{k}sb")
            nc.sync.dma_start(out=t[:], in_=W_d[k][:])
            W_sb.append(t)
        vec_sb = {}
        for n in vec_names:
            t = const.tile([128, 1], F32, tag=f"v_{n}", name=f"v_{n}")
            nc.sync.dma_start(out=t[:], in_=vec_d[n][:])
            vec_sb[n] = t
        dinv_src_sb = const.tile([128, NCHK], F32, tag="dsrc")
        nc.sync.dma_start(out=dinv_src_sb[:], in_=dinv_src_d[:])
        dslot_sb = const.tile([128, TOT_TILES], F32, tag="dslot")
        nc.sync.dma_start(out=dslot_sb[:], in_=dslot_d[:])
        pool_idx_sb = const.tile([128, PGRID // 16], I16, tag="pidx")
        nc.sync.dma_start(out=pool_idx_sb[:], in_=pool_idx_d[:])
        iota_f = const.tile([128, 128], F32, tag="iotaf")
        nc.sync.dma_start(out=iota_f[:], in_=iota_d[:])
        if cfg["msg_dt"] == "float32":
            iota_sb = iota_f
            dslot_m = dslot_sb
        else:
            iota_sb = const.tile([128, 128], MSG, tag="iotam")
            nc.vector.tensor_copy(out=iota_sb[:], in_=iota_f[:])
            dslot_m = const.tile([128, TOT_TILES], MSG, tag="dslotm")
            nc.vector.tensor_copy(out=dslot_m[:], in_=dslot_sb[:])
        eps_sb = const.tile([128, 1], F32, tag="eps")
        nc.vector.memset(eps_sb[:], eps)
        zlhs = const.tile([128, 128], MSG, tag="zlhs")
        nc.vector.memset(zlhs[:], 0)
        zrhs = const.tile([128, WIN], MSG, tag="zrhs")
        nc.vector.memset(zrhs[:], 0)

        # ---- DRAM scratch ----------------------------------------------
        y_self = dram.tile([P, 128], MSG, tag="yself")
        y_grp = [dram2.tile([GSZ, 128], MSG, tag=f"ygrp{g}", name=f"ygrp{g}")
                 for g in range(GRP)]
        h_nm = dram.tile([P + 1, 128], F16, tag="hnm")
        stats_in = dram2.tile([128, 2], F32, tag="st_in")
        stats_out = dram2.tile([128, 2], F32, tag="st_out")

        padrow = const.tile([1, 128], F16, tag="padrow")
        nc.vector.memset(padrow[:], PAD_NEG)
        nc.sync.dma_start(out=h_nm[P:P + 1, :], in_=padrow[:])

        h_prev = prepool.tile([128, P], F32, tag="pre", name="h0")
        nc.sync.dma_start(out=h_prev[:], in_=h0T_d[:])

        # flattened static schedule per (w, g): list of (tile_idx, c0)
        cells = []
        for w in range(NWIN):
            row = []
            for g in range(GRP):
                lst = []
                for b in range(NBLK):
                    nt = int(sched[g][w][b])
                    if nt == 0:
                        continue
                    t0 = int(tile_base[w][g][b])
                    c0 = (b - (w * WIN) // BLK) * BLK
                    for t in range(nt):
                        lst.append((t0 + t, c0))
                row.append(lst)
            cells.append(row)

        for k in range(cfg['nconv']):
            # ------------ A: y = dinv * (h_prev @ W) ---------------------
            for i in range(NCHK):
                cs = min(128, P - 128 * i)
                pt = ps_y.tile([128, 128], F32, tag="psy", name="psy")
                nc.tensor.matmul(out=pt[:cs, :],
                                 lhsT=h_prev[:, 128 * i:128 * i + cs],
                                 rhs=W_sb[k][:], start=True, stop=True)
                yt = ystg.tile([128, 128], MSG, tag="yt", name="yt")
                nc.vector.tensor_scalar_mul(
                    out=yt[:cs, :], in0=pt[:cs, :],
                    scalar1=dinv_src_sb[:cs, i:i + 1])
                nc.sync.dma_start(out=y_self[128 * i:128 * i + cs, :],
                                  in_=yt[:cs, :])

            # ------------ B: chunked AllGather ---------------------------
            for g in range(GRP):
                nc.gpsimd.collective_compute(
                    "AllGather", mybir.AluOpType.bypass, replica_groups=rg,
                    ins=[y_self[QTR * g:QTR * (g + 1), :]],
                    outs=[y_grp[g][:]])

            # ------------ C: aggregation ---------------------------------
            pre = prepool.tile([128, P], F32, tag="pre", name=f"pre{k}")
            if cfg["skip_agg"]:
                nc.vector.memset(pre[:], 1.0)
            for w in range(NWIN if not cfg["skip_agg"] else 0):
                wn = min(WIN, P - WIN * w)
                ps = ps_agg.tile([128, WIN], F32, tag="psagg", name="psagg")
                nc.tensor.matmul(out=ps[:, :], lhsT=zlhs[:], rhs=zrhs[:],
                                 start=True, stop=False,
                                 skip_group_check=True)
                dwin = dwp.tile([128, WIN], F32, tag="dwin", name="dwin")
                nc.sync.dma_start(
                    out=dwin[:, :wn],
                    in_=bcast_p(dinv_dst_d[:1, WIN * w:WIN * w + wn], 128))
                for g in range(GRP):
                    lst = cells[w][g]
                    for s0 in range(0, len(lst), SUBT):
                        sub = lst[s0:s0 + SUBT]
                        ct = len(sub)
                        t0 = sub[0][0]
                        ixt = ixp.tile([128, SUBT * 8], I16, tag="ixt",
                                       name="ixt")
                        nc.sync.dma_start(
                            out=ixt[:, :ct * 8],
                            in_=idx_d[:, t0 * 8:(t0 + ct) * 8])
                        mt = msgp.tile([128, SUBT, 128], MSG, tag="mt",
                                       name="mt")
                        nc.gpsimd.dma_gather(
                            out_ap=mt[:, :ct, :], in_ap=y_grp[g][:],
                            idxs_ap=ixt[:, :ct * 8],
                            num_idxs=ct * 128, num_idxs_reg=ct * 128,
                            elem_size=128, single_packet=False)
                        st = spp.tile([128, SUBT * 128], MSG, tag="st",
                                      name="st")
                        nc.vector.tensor_tensor(
                            out=st[:, :ct * 128].rearrange(
                                "p (t j) -> p t j", t=ct),
                            in0=dslot_m[:, t0:t0 + ct, None].to_broadcast(
                                [128, ct, 128]),
                            in1=iota_sb[:, None, :].to_broadcast(
                                [128, ct, 128]),
                            op=mybir.AluOpType.is_equal)
                        for ti, (_, c0) in enumerate(sub):
                            nc.tensor.matmul(
                                out=ps[:, c0:c0 + 128],
                                lhsT=mt[:, ti, :],
                                rhs=st[:, ti * 128:(ti + 1) * 128],
                                start=False, stop=False,
                                skip_group_check=True)
                nc.vector.tensor_tensor(
                    out=pre[:, WIN * w:WIN * w + wn],
                    in0=ps[:, :wn], in1=dwin[:, :wn],
                    op=mybir.AluOpType.mult)

            # ------------ D: BN + relu (in place on pre) -----------------
            if k < 2 and not cfg["skip_bn"]:
                nsub = _cdiv(P, 512)
                stats = small.tile([128, nsub, 6], F32, tag="bnstats",
                                   name="bnstats")
                for i in range(nsub):
                    cn = min(512, P - 512 * i)
                    nc.vector.bn_stats(out=stats[:, i, :],
                                       in_=pre[:, 512 * i:512 * i + cn])
                mv = small.tile([128, 2], F32, tag="mv", name="mv")
                nc.vector.bn_aggr(out=mv[:], in_=stats[:])
                loc = small.tile([128, 2], F32, tag="loc", name="loc")
                nc.vector.tensor_scalar_mul(out=loc[:, 0:1], in0=mv[:, 0:1],
                                            scalar1=float(P))
                nc.vector.tensor_tensor(out=loc[:, 1:2], in0=mv[:, 0:1],
                                        in1=mv[:, 0:1],
                                        op=mybir.AluOpType.mult)
                nc.vector.tensor_tensor(out=loc[:, 1:2], in0=loc[:, 1:2],
                                        in1=mv[:, 1:2],
                                        op=mybir.AluOpType.add)
                nc.vector.tensor_scalar_mul(out=loc[:, 1:2], in0=loc[:, 1:2],
                                            scalar1=float(P))
                nc.sync.dma_start(out=stats_in[:], in_=loc[:])
                nc.gpsimd.collective_compute(
                    "AllReduce", mybir.AluOpType.add, replica_groups=rg,
                    ins=[stats_in[:]], outs=[stats_out[:]])
                gl = small.tile([128, 2], F32, tag="gl", name="gl")
                nc.sync.dma_start(out=gl[:], in_=stats_out[:])
                mean = small.tile([128, 1], F32, tag="mean", name="mean")
                var = small.tile([128, 1], F32, tag="var", name="var")
                nc.vector.tensor_scalar_mul(out=mean[:], in0=gl[:, 0:1],
                                            scalar1=1.0 / N)
                nc.vector.tensor_scalar_mul(out=var[:], in0=gl[:, 1:2],
                                            scalar1=1.0 / N)
                m2 = small.tile([128, 1], F32, tag="m2", name="m2")
                nc.vector.tensor_tensor(out=m2[:], in0=mean[:], in1=mean[:],
                                        op=mybir.AluOpType.mult)
                nc.vector.tensor_tensor(out=var[:], in0=var[:], in1=m2[:],
                                        op=mybir.AluOpType.subtract)
                sd = small.tile([128, 1], F32, tag="sd", name="sd")
                nc.scalar.activation(out=sd[:], in_=var[:],
                                     func=mybir.ActivationFunctionType.Sqrt,
                                     bias=eps_sb[:], scale=1.0)
                rstd = small.tile([128, 1], F32, tag="rstd", name="rstd")
                nc.vector.reciprocal(out=rstd[:], in_=sd[:])
                gname, bname = ("g1", "be1") if k == 0 else ("g2", "be2")
                A = small.tile([128, 1], F32, tag="A", name="A")
                B = small.tile([128, 1], F32, tag="B", name="B")
                nc.vector.tensor_tensor(out=A[:], in0=rstd[:],
                                        in1=vec_sb[gname][:],
                                        op=mybir.AluOpType.mult)
                nc.vector.tensor_tensor(out=B[:], in0=mean[:], in1=A[:],
                                        op=mybir.AluOpType.mult)
                nc.vector.tensor_tensor(out=B[:], in0=vec_sb[bname][:],
                                        in1=B[:], op=mybir.AluOpType.subtract)
                nc.vector.tensor_scalar(out=pre[:], in0=pre[:],
                                        scalar1=A[:], scalar2=B[:],
                                        op0=mybir.AluOpType.mult,
                                        op1=mybir.AluOpType.add)
            elif k == 2:
                nc.vector.tensor_scalar_add(out=pre[:], in0=pre[:],
                                            scalar1=vec_sb["b3"][:])
            nc.vector.tensor_scalar_max(out=pre[:], in0=pre[:], scalar1=0.0)
            h_cur = pre

            # ------------ E: pooling -------------------------------------
            if cfg["skip_pool"]:
                h_prev = pre
                continue
            for i in range(NCHK):
                cs = min(128, P - 128 * i)
                pt = ps_tr.tile([128, 128], F32, tag="pstr", name="pstr")
                nc.tensor.transpose(out=pt[:cs, :],
                                    in_=h_cur[:, 128 * i:128 * i + cs],
                                    identity=ident[:])
                tt = trp.tile([128, 128], F16, tag="tt", name="tt")
                nc.vector.tensor_copy(out=tt[:cs, :], in_=pt[:cs, :])
                nc.sync.dma_start(out=h_nm[128 * i:128 * i + cs, :],
                                  in_=tt[:cs, :])
            pr = small.tile([128, SLOC], F32, tag="pr", name="pr")
            for ci in range(NPCH):
                csg = min(CSEG, SLOC - CSEG * ci)
                pg = pgp.tile([128, PC], F16, tag="pg", name="pg")
                nc.gpsimd.dma_gather(
                    out_ap=pg[:].rearrange("p (a n) -> p a n", a=1),
                    in_ap=h_nm[:],
                    idxs_ap=pool_idx_sb[:, ci * PC // 16:(ci + 1) * PC // 16],
                    num_idxs=PC, num_idxs_reg=PC, elem_size=128,
                    transpose=True, single_packet=False)
                nc.vector.tensor_reduce(
                    out=pr[:, CSEG * ci:CSEG * ci + csg],
                    in_=pg[:, :csg * L].rearrange("p (s l) -> p s l", s=csg),
                    axis=mybir.AxisListType.X, op=mybir.AluOpType.max)
            nc.sync.dma_start(out=out_d[k, :, :], in_=pr[:])

            h_prev = h_cur

    nc.finalize()
    return nc


# -------------------------------------------------------------------------
# entry point
# -------------------------------------------------------------------------
_CACHE = {}


def kernel(x, edge_index, node_to_subgraph, W1, b1, g1, be1, W2, b2, g2, be2,
           W3, b3, cfg=None, runner=None):
    cfg = dict(CFG, **(cfg or {}))
    NC = cfg["ncores"]; S = cfg["S"]
    meta, per_core = host_prep(x, edge_index, node_to_subgraph, cfg)

    key = (cfg["N"], cfg["S"], cfg["msg_dt"], meta["TOT_TILES"],
           meta["SLOC"], meta["L"], cfg["nconv"], cfg["skip_bn"],
           cfg["skip_pool"], cfg["skip_agg"], meta["sched"].tobytes())
    if key not in _CACHE:
        _CACHE.clear()
        _CACHE[key] = build_program(meta, cfg)
    nc = _CACHE[key]

    iota = np.tile(np.arange(128, dtype=np.float32), (128, 1))
    in_maps = []
    for c in range(NC):
        in_maps.append(dict(
            h0T=np.ascontiguousarray(per_core["h0T"][c], dtype=np.float32),
            W1=np.asarray(W1, np.float32), W2=np.asarray(W2, np.float32),
            W3=np.asarray(W3, np.float32),
            g1=np.asarray(g1, np.float32).reshape(-1, 1),
            be1=np.asarray(be1, np.float32).reshape(-1, 1),
            g2=np.asarray(g2, np.float32).reshape(-1, 1),
            be2=np.asarray(be2, np.float32).reshape(-1, 1),
            b3=np.asarray(b3, np.float32).reshape(-1, 1),
            dinv_src=per_core["dinv_src"][c],
            dinv_dst=np.ascontiguousarray(per_core["dinv_dst"][c]),
            idx=np.ascontiguousarray(per_core["idx"][c]),
            dslot=np.ascontiguousarray(per_core["dslot"][c]),
            pool_idx=np.ascontiguousarray(per_core["pool_idx"][c]),
            iota=iota,
        ))

    if runner is None:
        res = run_bass_kernel_spmd(nc, in_maps, list(range(NC)))
        results = res.results
    else:
        results = runner(nc, in_maps)

    out = np.full((S, 3 * D), -np.inf, dtype=np.float32)
    for c in range(NC):
        sf = int(meta["s_first"][c]); ns = int(meta["nseg"][c])
        po = results[c]["pool_out"]
        for k in range(3):
            out[sf:sf + ns, 128 * k:128 * (k + 1)] = np.maximum(
                out[sf:sf + ns, 128 * k:128 * (k + 1)], po[k, :, :ns].T)
    out[meta["seg_counts"] == 0, :] = -np.inf
    return out
